# revision 5
# baseline (speedup 1.0000x reference)
"""Block-sparse attention kernel for TRN2 (8 NeuronCores, 1 head per core).

Problem: q,k,v [1, 4096, 8, 128] f32, block_mask [64,64] bool with pattern
  causal & (2-block sliding window | vertical stripe on blocks {0,1}).
Masking is block-granular (mask expanded by repeat), so active blocks are
fully dense.

Per-core strategy (one head). The host prepares fp16 operands (the kernel
computes in fp16 regardless — same numerics, half the load traffic):
  qT, kT: [128, 4096] transposed,  vt: [128, 32*129] pre-tiled V with a
  ones-column per 128-row tile, so P^T @ [V | 1] accumulates both O and
  the softmax denominators in one matmul chain.

Scores are computed TRANSPOSED (ST[k, q] = K @ Q^T) so exp(ST) directly
yields P^T — the stationary operand PV needs. No PE transposes at all.

Banded scores are shared: ST_m (k blocks {2m, 2m+1} x 256 q) serves pair
m (its sliding window) and pair m+1 (its trailing window); invalid
(k-block, q-block) corners are memset to -1e30 before the exp.
The vertical stripe k{0,1} is computed for 512 q at a time (N=512 mm).
Softmax skips max-subtraction: scores*scale ~ N(0,1), exp is safe.
"""
import sys

if '/opt/trn_rl_repo' not in sys.path:
    sys.path.insert(0, '/opt/trn_rl_repo')

import numpy as np

SEQ = 4096
D = 128
BLOCK = 64
NBLK = SEQ // BLOCK
TILES = SEQ // 128           # 32 q-pair iterations
GROUPS = TILES // 4          # 8 vertical-score groups
STORE_W = 4                  # iterations per output store
N_CORES = 8
N_HEADS = 8
SCALE = 1.0 / float(np.sqrt(D))
NEG = -1e30
VW = 129                     # V tile width incl ones column


def _expected_block_mask():
    q = np.arange(NBLK)[:, None]
    k = np.arange(NBLK)[None, :]
    causal = q >= k
    sliding = (q - k) < 2
    vert = np.zeros(NBLK, dtype=bool)
    vert[0:2] = True
    return causal & (sliding | vert[None, :])


_CACHED_NC = None


def _build_nc():
    import concourse.bass as bass
    import concourse.bacc as bacc
    import concourse.tile as tile
    import concourse.mybir as mybir

    f32 = mybir.dt.float32
    f16 = mybir.dt.float16
    Exp = mybir.ActivationFunctionType.Exp

    nc = bacc.Bacc(None, target_bir_lowering=False)

    qt_d = nc.dram_tensor("qT", [D, SEQ], f16, kind="ExternalInput")
    kt_d = nc.dram_tensor("kT", [D, SEQ], f16, kind="ExternalInput")
    v_d = nc.dram_tensor("vt", [D, TILES * VW], f16, kind="ExternalInput")
    o_d = nc.dram_tensor("o", [SEQ, D], f32, kind="ExternalOutput")

    with tile.TileContext(nc) as tc:
        with tc.tile_pool(name="singles", bufs=1) as singles, \
             tc.tile_pool(name="ptv_pool", bufs=2) as ptv_pool, \
             tc.tile_pool(name="pts_pool", bufs=3) as pts_pool, \
             tc.tile_pool(name="sums", bufs=4) as sums, \
             tc.tile_pool(name="o_pool", bufs=2) as o_pool, \
             tc.tile_pool(name="stv_ps", bufs=2, space="PSUM") as stv_ps, \
             tc.tile_pool(name="st_ps", bufs=3, space="PSUM") as st_ps, \
             tc.tile_pool(name="o_ps", bufs=3, space="PSUM") as o_ps:

            # chunked loads: every matmul read below falls inside one
            # chunk tile, and compute can start after the first chunks.
            # qt chunks overlap by 128 cols (band rhs spans [128t,128t+256)).
            kt_tiles, qt_tiles, vb_tiles = [], [], []
            for c in range(GROUPS):
                ktile = singles.tile([128, 512], f16, name=f"kt_{c}", tag=f"kt{c}")
                nc.sync.dma_start(out=ktile[:], in_=kt_d[:, 512 * c:512 * c + 512])
                kt_tiles.append(ktile)
                qw_c = min(640, SEQ - 512 * c)
                qtile = singles.tile([128, qw_c], f16, name=f"qt_{c}", tag=f"qt{c}")
                nc.sync.dma_start(out=qtile[:], in_=qt_d[:, 512 * c:512 * c + qw_c])
                qt_tiles.append(qtile)
                vtile = singles.tile([128, 4 * VW], f16, name=f"vb_{c}", tag=f"vb{c}")
                nc.sync.dma_start(out=vtile[:],
                                  in_=v_d[:, 4 * VW * c:4 * VW * c + 4 * VW])
                vb_tiles.append(vtile)

            def ktc(col, width):
                return kt_tiles[col // 512][:, col % 512:col % 512 + width]

            def qtc(col, width):
                c = col // 512
                return qt_tiles[c][:, col - 512 * c:col - 512 * c + width]

            def vbt(t):
                return vb_tiles[t // 4][:, VW * (t % 4):VW * (t % 4) + VW]

            pt_tiles = [None] * TILES
            ptv = None
            osb = None

            for t in range(TILES):
                g, j = divmod(t, 4)

                # ---- vertical stripe scores, once per 4 iterations ----
                if j == 0:
                    stv = stv_ps.tile([128, 512], f32, tag="stv")
                    nc.tensor.matmul(stv[:], ktc(0, 128), qtc(512 * g, 512),
                                     start=True, stop=True)
                    if g == 0:
                        # query block 0 must not see key block 1
                        nc.vector.memset(stv[64:128, 0:64], NEG)
                    ptv = ptv_pool.tile([128, 512], f16, tag="ptv")
                    nc.scalar.activation(ptv[:], stv[:], Exp,
                                         scale=float(SCALE))
                qv = slice(128 * j, 128 * j + 128)

                # ---- banded scores ST_t: k blocks {2t, 2t+1} ----
                # q columns [128t, 128t+256): this pair's sliding window
                # plus the next pair's trailing window.
                if t >= 1:
                    qw = min(256, SEQ - 128 * t)
                    st = st_ps.tile([128, 256], f32, tag="st")
                    nc.tensor.matmul(st[:, 0:qw], ktc(128 * t, 128),
                                     qtc(128 * t, qw),
                                     start=True, stop=True)
                    if qw == 256:
                        # k block 2t invisible to pair t+1 (both halves)
                        nc.vector.memset(st[0:64, 128:256], NEG)
                        # k block 2t+1: invisible to q blocks 2t and 2t+3
                        sta = st[:]
                        m2 = bass.AP(tensor=sta.tensor,
                                     offset=sta.offset + 64 * sta.ap[0][0],
                                     ap=[[sta.ap[0][0], 64], [192, 2], [1, 64]])
                        nc.vector.memset(m2, NEG)
                    else:
                        nc.vector.memset(st[64:128, 0:64], NEG)
                    pts = pts_pool.tile([128, 256], f16, tag="pts")
                    nc.scalar.activation(pts[:, 0:qw], st[:, 0:qw], Exp,
                                         scale=float(SCALE))
                    pt_tiles[t] = pts

                # ---- PV: O'[q, 0:128]=O, O'[q, 128]=denominator ----
                ov = o_ps.tile([128, VW], f32, tag="ov")
                nmm = 1 + (1 if t >= 1 else 0) + (1 if t >= 2 else 0)
                nc.tensor.matmul(ov[:], ptv[:, qv], vbt(0),
                                 start=True, stop=(nmm == 1))
                if t >= 2:
                    nc.tensor.matmul(ov[:], pt_tiles[t - 1][:, 128:256],
                                     vbt(t - 1), start=False, stop=False)
                if t >= 1:
                    nc.tensor.matmul(ov[:], pt_tiles[t][:, 0:128],
                                     vbt(t), start=False, stop=True)

                # ---- normalize; store every STORE_W iterations ----
                sj = t % STORE_W
                if sj == 0:
                    osb = o_pool.tile([128, 128 * STORE_W], f32, tag="osb")
                recip = sums.tile([128, 1], f32, tag="recip")
                nc.vector.reciprocal(recip[:], ov[:, 128:129])
                nc.vector.tensor_scalar_mul(osb[:, 128 * sj:128 * sj + 128],
                                            ov[:, 0:128], recip[:])
                if sj == STORE_W - 1:
                    t0 = t - STORE_W + 1
                    oap = bass.AP(tensor=o_d[:].tensor,
                                  offset=128 * t0 * 128,
                                  ap=[[128, 128], [128 * 128, STORE_W],
                                      [1, 128]])
                    nc.sync.dma_start(out=oap, in_=osb[:])

    nc.compile()
    return nc


def _get_nc():
    global _CACHED_NC
    if _CACHED_NC is None:
        _CACHED_NC = _build_nc()
    return _CACHED_NC


def _run(inputs, trace=False, trace_kwargs=None):
    import ml_dtypes
    from concourse.bass_utils import run_bass_kernel_spmd

    q, k, v = inputs["q"], inputs["k"], inputs["v"]
    block_mask = np.asarray(inputs["block_mask"])
    assert np.array_equal(block_mask, _expected_block_mask()), \
        "kernel compiled for the DKernel predefined sparse pattern only"

    nc = _get_nc()
    f16 = ml_dtypes.float16 if hasattr(ml_dtypes, "float16") else np.float16
    in_maps = []
    for h in range(N_CORES):
        qh = np.asarray(q[0, :, h, :], dtype=np.float32)
        kh = np.asarray(k[0, :, h, :], dtype=np.float32)
        vh = np.asarray(v[0, :, h, :], dtype=np.float32)
        # pre-tiled [V | 1] in [128, 32*129] layout: tile t holds V rows
        # [128t, 128t+128) with a trailing ones column
        vt = np.ones((128, TILES * VW), dtype=np.float16)
        vr = vh.astype(np.float16).reshape(TILES, 128, D)
        for t in range(TILES):
            vt[:, VW * t:VW * t + 128] = vr[t]
        in_maps.append({
            "qT": np.ascontiguousarray(qh.T.astype(np.float16)),
            "kT": np.ascontiguousarray(kh.T.astype(np.float16)),
            "vt": vt,
        })
    kwargs = {}
    if trace:
        kwargs["trace"] = True
        if trace_kwargs:
            kwargs.update(trace_kwargs)
    res = run_bass_kernel_spmd(nc, in_maps, list(range(N_CORES)), **kwargs)
    out = np.empty((1, SEQ, N_HEADS, D), dtype=np.float32)
    for h in range(N_CORES):
        out[0, :, h, :] = res.results[h]["o"]
    return out, res


def kernel(q, k, v, block_mask):
    out, _ = _run({"q": q, "k": k, "v": v, "block_mask": block_mask})
    return out


# revision 6
# speedup vs baseline: 1.0193x; 1.0193x over previous
"""Block-sparse attention kernel for TRN2 (8 NeuronCores, 1 head per core).

Problem: q,k,v [1, 4096, 8, 128] f32, block_mask [64,64] bool with pattern
  causal & (2-block sliding window | vertical stripe on blocks {0,1}).
Masking is block-granular (mask expanded by repeat), so active blocks are
fully dense.

Per-core strategy (one head). The host prepares fp16 operands (the kernel
computes in fp16 regardless — same numerics, half the load traffic):
  qT, kT: [128, 4096] transposed,  vt: [128, 32*129] pre-tiled V with a
  ones-column per 128-row tile, so P^T @ [V | 1] accumulates both O and
  the softmax denominators in one matmul chain.

Scores are computed TRANSPOSED (ST[k, q] = K @ Q^T) so exp(ST) directly
yields P^T — the stationary operand PV needs. No PE transposes at all.

Banded scores are shared: ST_m (k blocks {2m, 2m+1} x 256 q) serves pair
m (its sliding window) and pair m+1 (its trailing window); invalid
(k-block, q-block) corners are memset to -1e30 before the exp.
The vertical stripe k{0,1} is computed for 512 q at a time (N=512 mm).
Softmax skips max-subtraction: scores*scale ~ N(0,1), exp is safe.
"""
import sys

if '/opt/trn_rl_repo' not in sys.path:
    sys.path.insert(0, '/opt/trn_rl_repo')

import numpy as np

SEQ = 4096
D = 128
BLOCK = 64
NBLK = SEQ // BLOCK
TILES = SEQ // 128           # 32 q-pair iterations
GROUPS = TILES // 4          # 8 vertical-score groups
STORE_W = 4                  # iterations per output store
N_CORES = 8
N_HEADS = 8
SCALE = 1.0 / float(np.sqrt(D))
NEG = -1e30
VW = 129                     # V tile width incl ones column


def _expected_block_mask():
    q = np.arange(NBLK)[:, None]
    k = np.arange(NBLK)[None, :]
    causal = q >= k
    sliding = (q - k) < 2
    vert = np.zeros(NBLK, dtype=bool)
    vert[0:2] = True
    return causal & (sliding | vert[None, :])


_CACHED_NC = None


def _build_nc():
    import concourse.bass as bass
    import concourse.bacc as bacc
    import concourse.tile as tile
    import concourse.mybir as mybir

    f32 = mybir.dt.float32
    f16 = mybir.dt.float16
    Exp = mybir.ActivationFunctionType.Exp

    nc = bacc.Bacc(None, target_bir_lowering=False)

    qt_d = nc.dram_tensor("qT", [D, SEQ], f16, kind="ExternalInput")
    kt_d = nc.dram_tensor("kT", [D, SEQ], f16, kind="ExternalInput")
    v_d = nc.dram_tensor("vt", [D, TILES * VW], f16, kind="ExternalInput")
    o_d = nc.dram_tensor("o", [SEQ, D], f32, kind="ExternalOutput")

    with tile.TileContext(nc) as tc:
        with tc.tile_pool(name="singles", bufs=1) as singles, \
             tc.tile_pool(name="ptv_pool", bufs=GROUPS) as ptv_pool, \
             tc.tile_pool(name="pts_pool", bufs=3) as pts_pool, \
             tc.tile_pool(name="sums", bufs=4) as sums, \
             tc.tile_pool(name="o_pool", bufs=2) as o_pool, \
             tc.tile_pool(name="stv_ps", bufs=2, space="PSUM") as stv_ps, \
             tc.tile_pool(name="st_ps", bufs=3, space="PSUM") as st_ps, \
             tc.tile_pool(name="o_ps", bufs=3, space="PSUM") as o_ps:

            # chunked loads: every matmul read below falls inside one
            # chunk tile, and compute can start after the first chunks.
            # qt chunks overlap by 128 cols (band rhs spans [128t,128t+256)).
            kt_tiles, qt_tiles, vb_tiles = [], [], []
            for c in range(GROUPS):
                ktile = singles.tile([128, 512], f16, name=f"kt_{c}", tag=f"kt{c}")
                nc.sync.dma_start(out=ktile[:], in_=kt_d[:, 512 * c:512 * c + 512])
                kt_tiles.append(ktile)
                qw_c = min(640, SEQ - 512 * c)
                qtile = singles.tile([128, qw_c], f16, name=f"qt_{c}", tag=f"qt{c}")
                nc.sync.dma_start(out=qtile[:], in_=qt_d[:, 512 * c:512 * c + qw_c])
                qt_tiles.append(qtile)
                vtile = singles.tile([128, 4 * VW], f16, name=f"vb_{c}", tag=f"vb{c}")
                nc.sync.dma_start(out=vtile[:],
                                  in_=v_d[:, 4 * VW * c:4 * VW * c + 4 * VW])
                vb_tiles.append(vtile)

            def ktc(col, width):
                return kt_tiles[col // 512][:, col % 512:col % 512 + width]

            def qtc(col, width):
                c = col // 512
                return qt_tiles[c][:, col - 512 * c:col - 512 * c + width]

            def vbt(t):
                return vb_tiles[t // 4][:, VW * (t % 4):VW * (t % 4) + VW]

            # ---- all vertical-stripe scores up front (pipeline depth) ----
            ptvs = []
            for g in range(GROUPS):
                stv = stv_ps.tile([128, 512], f32, tag="stv")
                nc.tensor.matmul(stv[:], ktc(0, 128), qtc(512 * g, 512),
                                 start=True, stop=True)
                if g == 0:
                    # query block 0 must not see key block 1
                    nc.vector.memset(stv[64:128, 0:64], NEG)
                ptv_g = ptv_pool.tile([128, 512], f16, tag="ptv")
                nc.scalar.activation(ptv_g[:], stv[:], Exp,
                                     scale=float(SCALE))
                ptvs.append(ptv_g)

            pt_tiles = [None] * TILES
            osb = None

            for t in range(TILES):
                g, j = divmod(t, 4)
                ptv = ptvs[g]
                qv = slice(128 * j, 128 * j + 128)

                # ---- banded scores ST_t: k blocks {2t, 2t+1} ----
                # q columns [128t, 128t+256): this pair's sliding window
                # plus the next pair's trailing window.
                if t >= 1:
                    qw = min(256, SEQ - 128 * t)
                    st = st_ps.tile([128, 256], f32, tag="st")
                    nc.tensor.matmul(st[:, 0:qw], ktc(128 * t, 128),
                                     qtc(128 * t, qw),
                                     start=True, stop=True)
                    if qw == 256:
                        # k block 2t invisible to pair t+1 (both halves)
                        nc.vector.memset(st[0:64, 128:256], NEG)
                        # k block 2t+1: invisible to q blocks 2t and 2t+3
                        sta = st[:]
                        m2 = bass.AP(tensor=sta.tensor,
                                     offset=sta.offset + 64 * sta.ap[0][0],
                                     ap=[[sta.ap[0][0], 64], [192, 2], [1, 64]])
                        nc.vector.memset(m2, NEG)
                    else:
                        nc.vector.memset(st[64:128, 0:64], NEG)
                    pts = pts_pool.tile([128, 256], f16, tag="pts")
                    nc.scalar.activation(pts[:, 0:qw], st[:, 0:qw], Exp,
                                         scale=float(SCALE))
                    pt_tiles[t] = pts

                # ---- PV: O'[q, 0:128]=O, O'[q, 128]=denominator ----
                ov = o_ps.tile([128, VW], f32, tag="ov")
                nmm = 1 + (1 if t >= 1 else 0) + (1 if t >= 2 else 0)
                nc.tensor.matmul(ov[:], ptv[:, qv], vbt(0),
                                 start=True, stop=(nmm == 1))
                if t >= 2:
                    nc.tensor.matmul(ov[:], pt_tiles[t - 1][:, 128:256],
                                     vbt(t - 1), start=False, stop=False)
                if t >= 1:
                    nc.tensor.matmul(ov[:], pt_tiles[t][:, 0:128],
                                     vbt(t), start=False, stop=True)

                # ---- normalize; store every STORE_W iterations ----
                sj = t % STORE_W
                if sj == 0:
                    osb = o_pool.tile([128, 128 * STORE_W], f32, tag="osb")
                recip = sums.tile([128, 1], f32, tag="recip")
                nc.vector.reciprocal(recip[:], ov[:, 128:129])
                nc.vector.tensor_scalar_mul(osb[:, 128 * sj:128 * sj + 128],
                                            ov[:, 0:128], recip[:])
                if sj == STORE_W - 1:
                    t0 = t - STORE_W + 1
                    oap = bass.AP(tensor=o_d[:].tensor,
                                  offset=128 * t0 * 128,
                                  ap=[[128, 128], [128 * 128, STORE_W],
                                      [1, 128]])
                    nc.sync.dma_start(out=oap, in_=osb[:])

    nc.compile()
    return nc


def _get_nc():
    global _CACHED_NC
    if _CACHED_NC is None:
        _CACHED_NC = _build_nc()
    return _CACHED_NC


def _run(inputs, trace=False, trace_kwargs=None):
    import ml_dtypes
    from concourse.bass_utils import run_bass_kernel_spmd

    q, k, v = inputs["q"], inputs["k"], inputs["v"]
    block_mask = np.asarray(inputs["block_mask"])
    assert np.array_equal(block_mask, _expected_block_mask()), \
        "kernel compiled for the DKernel predefined sparse pattern only"

    nc = _get_nc()
    f16 = ml_dtypes.float16 if hasattr(ml_dtypes, "float16") else np.float16
    in_maps = []
    for h in range(N_CORES):
        qh = np.asarray(q[0, :, h, :], dtype=np.float32)
        kh = np.asarray(k[0, :, h, :], dtype=np.float32)
        vh = np.asarray(v[0, :, h, :], dtype=np.float32)
        # pre-tiled [V | 1] in [128, 32*129] layout: tile t holds V rows
        # [128t, 128t+128) with a trailing ones column
        vt = np.ones((128, TILES * VW), dtype=np.float16)
        vr = vh.astype(np.float16).reshape(TILES, 128, D)
        for t in range(TILES):
            vt[:, VW * t:VW * t + 128] = vr[t]
        in_maps.append({
            "qT": np.ascontiguousarray(qh.T.astype(np.float16)),
            "kT": np.ascontiguousarray(kh.T.astype(np.float16)),
            "vt": vt,
        })
    kwargs = {}
    if trace:
        kwargs["trace"] = True
        if trace_kwargs:
            kwargs.update(trace_kwargs)
    res = run_bass_kernel_spmd(nc, in_maps, list(range(N_CORES)), **kwargs)
    out = np.empty((1, SEQ, N_HEADS, D), dtype=np.float32)
    for h in range(N_CORES):
        out[0, :, h, :] = res.results[h]["o"]
    return out, res


def kernel(q, k, v, block_mask):
    out, _ = _run({"q": q, "k": k, "v": v, "block_mask": block_mask})
    return out


# revision 7
# speedup vs baseline: 1.1475x; 1.1258x over previous
"""Block-sparse attention kernel for TRN2 (8 NeuronCores, 1 head per core).

Problem: q,k,v [1, 4096, 8, 128] f32, block_mask [64,64] bool with pattern
  causal & (2-block sliding window | vertical stripe on blocks {0,1}).
Masking is block-granular (mask expanded by repeat), so active blocks are
fully dense.

Per-core strategy (one head). The host prepares fp16 operands (the kernel
computes in fp16 regardless — same numerics, half the load traffic):
  qT, kT: [128, 4096] transposed,  vt: [128, 32*129] pre-tiled V with a
  ones-column per 128-row tile, so P^T @ [V | 1] accumulates both O and
  the softmax denominators in one matmul chain.

Scores are computed TRANSPOSED (ST[k, q] = K @ Q^T) so exp(ST) directly
yields P^T — the stationary operand PV needs. No PE transposes at all.

Banded scores are shared: ST_m (k blocks {2m, 2m+1} x 256 q) serves pair
m (its sliding window) and pair m+1 (its trailing window); invalid
(k-block, q-block) corners are memset to -1e30 before the exp.
The vertical stripe k{0,1} is computed for 512 q at a time (N=512 mm).
Softmax skips max-subtraction: scores*scale ~ N(0,1), exp is safe.
"""
import sys

if '/opt/trn_rl_repo' not in sys.path:
    sys.path.insert(0, '/opt/trn_rl_repo')

import numpy as np

SEQ = 4096
D = 128
BLOCK = 64
NBLK = SEQ // BLOCK
TILES = SEQ // 128           # 32 q-pair iterations
GROUPS = TILES // 4          # 8 vertical-score groups
STORE_W = 4                  # iterations per output store
N_CORES = 8
N_HEADS = 8
SCALE = 1.0 / float(np.sqrt(D))
NEG = -1e30
VW = 129                     # V tile width incl ones column


def _expected_block_mask():
    q = np.arange(NBLK)[:, None]
    k = np.arange(NBLK)[None, :]
    causal = q >= k
    sliding = (q - k) < 2
    vert = np.zeros(NBLK, dtype=bool)
    vert[0:2] = True
    return causal & (sliding | vert[None, :])


_CACHED_NC = None


def _build_nc():
    import concourse.bass as bass
    import concourse.bacc as bacc
    import concourse.tile as tile
    import concourse.mybir as mybir

    f32 = mybir.dt.float32
    f16 = mybir.dt.float16
    Exp = mybir.ActivationFunctionType.Exp

    nc = bacc.Bacc(None, target_bir_lowering=False)

    qt_d = nc.dram_tensor("qT", [D, SEQ], f16, kind="ExternalInput")
    kt_d = nc.dram_tensor("kT", [D, SEQ], f16, kind="ExternalInput")
    v_d = nc.dram_tensor("vt", [D, TILES * VW], f16, kind="ExternalInput")
    o_d = nc.dram_tensor("o", [SEQ, D], f32, kind="ExternalOutput")

    with tile.TileContext(nc) as tc:
        with tc.tile_pool(name="singles", bufs=1) as singles, \
             tc.tile_pool(name="ptv_pool", bufs=GROUPS) as ptv_pool, \
             tc.tile_pool(name="pts_pool", bufs=3) as pts_pool, \
             tc.tile_pool(name="sums", bufs=4) as sums, \
             tc.tile_pool(name="o_pool", bufs=2) as o_pool, \
             tc.tile_pool(name="stv_ps", bufs=2, space="PSUM") as stv_ps, \
             tc.tile_pool(name="st_ps", bufs=3, space="PSUM") as st_ps, \
             tc.tile_pool(name="o_ps", bufs=3, space="PSUM") as o_ps:

            # chunked loads: every matmul read below falls inside one
            # chunk tile, and compute can start after the first chunks.
            # qt chunks overlap by 128 cols (band rhs spans [128t,128t+256)).
            kt_tiles, qt_tiles, vb_tiles = [], [], []
            for c in range(GROUPS):
                ktile = singles.tile([128, 512], f16, name=f"kt_{c}", tag=f"kt{c}")
                nc.sync.dma_start(out=ktile[:], in_=kt_d[:, 512 * c:512 * c + 512])
                kt_tiles.append(ktile)
                qw_c = min(640, SEQ - 512 * c)
                qtile = singles.tile([128, qw_c], f16, name=f"qt_{c}", tag=f"qt{c}")
                nc.sync.dma_start(out=qtile[:], in_=qt_d[:, 512 * c:512 * c + qw_c])
                qt_tiles.append(qtile)
                vtile = singles.tile([128, 4 * VW], f16, name=f"vb_{c}", tag=f"vb{c}")
                nc.sync.dma_start(out=vtile[:],
                                  in_=v_d[:, 4 * VW * c:4 * VW * c + 4 * VW])
                vb_tiles.append(vtile)

            def ktc(col, width):
                return kt_tiles[col // 512][:, col % 512:col % 512 + width]

            def qtc(col, width):
                c = col // 512
                return qt_tiles[c][:, col - 512 * c:col - 512 * c + width]

            def vbt(t):
                return vb_tiles[t // 4][:, VW * (t % 4):VW * (t % 4) + VW]

            # ---- all vertical-stripe scores up front (pipeline depth) ----
            ptvs = []
            for g in range(GROUPS):
                stv = stv_ps.tile([128, 512], f32, tag="stv")
                nc.tensor.matmul(stv[:], ktc(0, 128), qtc(512 * g, 512),
                                 start=True, stop=True)
                ptv_g = ptv_pool.tile([128, 512], f16, tag="ptv")
                nc.scalar.activation(ptv_g[:], stv[:], Exp,
                                     scale=float(SCALE))
                if g == 0:
                    # query block 0 must not see key block 1
                    nc.gpsimd.memset(ptv_g[64:128, 0:64], 0.0)
                ptvs.append(ptv_g)

            pt_tiles = [None] * TILES
            osb = None

            for t in range(TILES):
                g, j = divmod(t, 4)
                ptv = ptvs[g]
                qv = slice(128 * j, 128 * j + 128)

                # ---- banded scores ST_t: k blocks {2t, 2t+1} ----
                # q columns [128t, 128t+256): this pair's sliding window
                # plus the next pair's trailing window.
                if t >= 1:
                    qw = min(256, SEQ - 128 * t)
                    st = st_ps.tile([128, 256], f32, tag="st")
                    nc.tensor.matmul(st[:, 0:qw], ktc(128 * t, 128),
                                     qtc(128 * t, qw),
                                     start=True, stop=True)
                    pts = pts_pool.tile([128, 256], f16, tag="pts")
                    nc.scalar.activation(pts[:, 0:qw], st[:, 0:qw], Exp,
                                         scale=float(SCALE))
                    if qw == 256:
                        # k block 2t invisible to pair t+1 (both halves)
                        nc.gpsimd.memset(pts[0:64, 128:256], 0.0)
                        # k block 2t+1: invisible to q blocks 2t and 2t+3
                        pa = pts[:]
                        m2 = bass.AP(tensor=pa.tensor,
                                     offset=pa.offset + 64 * pa.ap[0][0],
                                     ap=[[pa.ap[0][0], 64], [192, 2], [1, 64]])
                        nc.gpsimd.memset(m2, 0.0)
                    else:
                        nc.gpsimd.memset(pts[64:128, 0:64], 0.0)
                    pt_tiles[t] = pts

                # ---- PV: O'[q, 0:128]=O, O'[q, 128]=denominator ----
                ov = o_ps.tile([128, VW], f32, tag="ov")
                nmm = 1 + (1 if t >= 1 else 0) + (1 if t >= 2 else 0)
                nc.tensor.matmul(ov[:], ptv[:, qv], vbt(0),
                                 start=True, stop=(nmm == 1))
                if t >= 2:
                    nc.tensor.matmul(ov[:], pt_tiles[t - 1][:, 128:256],
                                     vbt(t - 1), start=False, stop=False)
                if t >= 1:
                    nc.tensor.matmul(ov[:], pt_tiles[t][:, 0:128],
                                     vbt(t), start=False, stop=True)

                # ---- normalize; store every STORE_W iterations ----
                sj = t % STORE_W
                if sj == 0:
                    osb = o_pool.tile([128, 128 * STORE_W], f32, tag="osb")
                recip = sums.tile([128, 1], f32, tag="recip")
                nc.vector.reciprocal(recip[:], ov[:, 128:129])
                nc.vector.tensor_scalar_mul(osb[:, 128 * sj:128 * sj + 128],
                                            ov[:, 0:128], recip[:])
                if sj == STORE_W - 1:
                    t0 = t - STORE_W + 1
                    oap = bass.AP(tensor=o_d[:].tensor,
                                  offset=128 * t0 * 128,
                                  ap=[[128, 128], [128 * 128, STORE_W],
                                      [1, 128]])
                    nc.sync.dma_start(out=oap, in_=osb[:])

    nc.compile()
    return nc


def _get_nc():
    global _CACHED_NC
    if _CACHED_NC is None:
        _CACHED_NC = _build_nc()
    return _CACHED_NC


def _run(inputs, trace=False, trace_kwargs=None):
    import ml_dtypes
    from concourse.bass_utils import run_bass_kernel_spmd

    q, k, v = inputs["q"], inputs["k"], inputs["v"]
    block_mask = np.asarray(inputs["block_mask"])
    assert np.array_equal(block_mask, _expected_block_mask()), \
        "kernel compiled for the DKernel predefined sparse pattern only"

    nc = _get_nc()
    f16 = ml_dtypes.float16 if hasattr(ml_dtypes, "float16") else np.float16
    in_maps = []
    for h in range(N_CORES):
        qh = np.asarray(q[0, :, h, :], dtype=np.float32)
        kh = np.asarray(k[0, :, h, :], dtype=np.float32)
        vh = np.asarray(v[0, :, h, :], dtype=np.float32)
        # pre-tiled [V | 1] in [128, 32*129] layout: tile t holds V rows
        # [128t, 128t+128) with a trailing ones column
        vt = np.ones((128, TILES * VW), dtype=np.float16)
        vr = vh.astype(np.float16).reshape(TILES, 128, D)
        for t in range(TILES):
            vt[:, VW * t:VW * t + 128] = vr[t]
        in_maps.append({
            "qT": np.ascontiguousarray(qh.T.astype(np.float16)),
            "kT": np.ascontiguousarray(kh.T.astype(np.float16)),
            "vt": vt,
        })
    kwargs = {}
    if trace:
        kwargs["trace"] = True
        if trace_kwargs:
            kwargs.update(trace_kwargs)
    res = run_bass_kernel_spmd(nc, in_maps, list(range(N_CORES)), **kwargs)
    out = np.empty((1, SEQ, N_HEADS, D), dtype=np.float32)
    for h in range(N_CORES):
        out[0, :, h, :] = res.results[h]["o"]
    return out, res


def kernel(q, k, v, block_mask):
    out, _ = _run({"q": q, "k": k, "v": v, "block_mask": block_mask})
    return out


# revision 8
# speedup vs baseline: 1.2032x; 1.0485x over previous
"""Block-sparse attention kernel for TRN2 (8 NeuronCores, 1 head per core).

Problem: q,k,v [1, 4096, 8, 128] f32, block_mask [64,64] bool with pattern
  causal & (2-block sliding window | vertical stripe on blocks {0,1}).
Masking is block-granular (mask expanded by repeat), so active blocks are
fully dense.

Per-core strategy (one head). The host prepares fp16 operands (the kernel
computes in fp16 regardless — same numerics, half the load traffic):
  qT, kT: [128, 4096] transposed,  vt: [128, 32*129] pre-tiled V with a
  ones-column per 128-row tile, so P^T @ [V | 1] accumulates both O and
  the softmax denominators in one matmul chain.

Scores are computed TRANSPOSED (ST[k, q] = K @ Q^T) so exp(ST) directly
yields P^T — the stationary operand PV needs. No PE transposes at all.

Banded scores are shared: ST_m (k blocks {2m, 2m+1} x 256 q) serves pair
m (its sliding window) and pair m+1 (its trailing window); invalid
(k-block, q-block) corners are memset to -1e30 before the exp.
The vertical stripe k{0,1} is computed for 512 q at a time (N=512 mm).
Softmax skips max-subtraction: scores*scale ~ N(0,1), exp is safe.
"""
import sys

if '/opt/trn_rl_repo' not in sys.path:
    sys.path.insert(0, '/opt/trn_rl_repo')

import numpy as np

SEQ = 4096
D = 128
BLOCK = 64
NBLK = SEQ // BLOCK
TILES = SEQ // 128           # 32 q-pair iterations
GROUPS = TILES // 4          # 8 vertical-score groups
STORE_W = 8                  # iterations per output store
N_CORES = 8
N_HEADS = 8
SCALE = 1.0 / float(np.sqrt(D))
NEG = -1e30
VW = 129                     # V tile width incl ones column


def _expected_block_mask():
    q = np.arange(NBLK)[:, None]
    k = np.arange(NBLK)[None, :]
    causal = q >= k
    sliding = (q - k) < 2
    vert = np.zeros(NBLK, dtype=bool)
    vert[0:2] = True
    return causal & (sliding | vert[None, :])


_CACHED_NC = None


def _build_nc():
    import concourse.bass as bass
    import concourse.bacc as bacc
    import concourse.tile as tile
    import concourse.mybir as mybir

    f32 = mybir.dt.float32
    f16 = mybir.dt.float16
    Exp = mybir.ActivationFunctionType.Exp

    nc = bacc.Bacc(None, target_bir_lowering=False)

    qt_d = nc.dram_tensor("qT", [D, SEQ], f16, kind="ExternalInput")
    kt_d = nc.dram_tensor("kT", [D, SEQ], f16, kind="ExternalInput")
    v_d = nc.dram_tensor("vt", [D, TILES * VW], f16, kind="ExternalInput")
    o_d = nc.dram_tensor("o", [SEQ, D], f32, kind="ExternalOutput")

    with tile.TileContext(nc) as tc:
        with tc.tile_pool(name="singles", bufs=1) as singles, \
             tc.tile_pool(name="ptv_pool", bufs=GROUPS) as ptv_pool, \
             tc.tile_pool(name="pts_pool", bufs=4) as pts_pool, \
             tc.tile_pool(name="sums", bufs=8) as sums, \
             tc.tile_pool(name="o_pool", bufs=3) as o_pool, \
             tc.tile_pool(name="stv_ps", bufs=2, space="PSUM") as stv_ps, \
             tc.tile_pool(name="st_ps", bufs=3, space="PSUM") as st_ps, \
             tc.tile_pool(name="o_ps", bufs=3, space="PSUM") as o_ps:

            # chunked loads: every matmul read below falls inside one
            # chunk tile, and compute can start after the first chunks.
            # qt chunks overlap by 128 cols (band rhs spans [128t,128t+256)).
            kt_tiles, qt_tiles, vb_tiles = [], [], []
            for c in range(GROUPS):
                ktile = singles.tile([128, 512], f16, name=f"kt_{c}", tag=f"kt{c}")
                nc.sync.dma_start(out=ktile[:], in_=kt_d[:, 512 * c:512 * c + 512])
                kt_tiles.append(ktile)
                qw_c = min(640, SEQ - 512 * c)
                qtile = singles.tile([128, qw_c], f16, name=f"qt_{c}", tag=f"qt{c}")
                nc.sync.dma_start(out=qtile[:], in_=qt_d[:, 512 * c:512 * c + qw_c])
                qt_tiles.append(qtile)
                vtile = singles.tile([128, 4 * VW], f16, name=f"vb_{c}", tag=f"vb{c}")
                nc.sync.dma_start(out=vtile[:],
                                  in_=v_d[:, 4 * VW * c:4 * VW * c + 4 * VW])
                vb_tiles.append(vtile)

            def ktc(col, width):
                return kt_tiles[col // 512][:, col % 512:col % 512 + width]

            def qtc(col, width):
                c = col // 512
                return qt_tiles[c][:, col - 512 * c:col - 512 * c + width]

            def vbt(t):
                return vb_tiles[t // 4][:, VW * (t % 4):VW * (t % 4) + VW]

            # ---- all vertical-stripe scores up front (pipeline depth) ----
            ptvs = []
            for g in range(GROUPS):
                stv = stv_ps.tile([128, 512], f32, tag="stv")
                nc.tensor.matmul(stv[:], ktc(0, 128), qtc(512 * g, 512),
                                 start=True, stop=True)
                ptv_g = ptv_pool.tile([128, 512], f16, tag="ptv")
                nc.scalar.activation(ptv_g[:], stv[:], Exp,
                                     scale=float(SCALE))
                if g == 0:
                    # query block 0 must not see key block 1
                    nc.gpsimd.memset(ptv_g[64:128, 0:64], 0.0)
                ptvs.append(ptv_g)

            pt_tiles = [None] * TILES
            osb = None

            for t in range(TILES):
                g, j = divmod(t, 4)
                ptv = ptvs[g]
                qv = slice(128 * j, 128 * j + 128)

                # ---- banded scores ST_t: k blocks {2t, 2t+1} ----
                # q columns [128t, 128t+256): this pair's sliding window
                # plus the next pair's trailing window.
                if t >= 1:
                    qw = min(256, SEQ - 128 * t)
                    st = st_ps.tile([128, 256], f32, tag="st")
                    nc.tensor.matmul(st[:, 0:qw], ktc(128 * t, 128),
                                     qtc(128 * t, qw),
                                     start=True, stop=True)
                    pts = pts_pool.tile([128, 256], f16, tag="pts")
                    nc.scalar.activation(pts[:, 0:qw], st[:, 0:qw], Exp,
                                         scale=float(SCALE))
                    if qw == 256:
                        # k block 2t invisible to pair t+1 (both halves)
                        nc.gpsimd.memset(pts[0:64, 128:256], 0.0)
                        # k block 2t+1: invisible to q blocks 2t and 2t+3
                        pa = pts[:]
                        m2 = bass.AP(tensor=pa.tensor,
                                     offset=pa.offset + 64 * pa.ap[0][0],
                                     ap=[[pa.ap[0][0], 64], [192, 2], [1, 64]])
                        nc.gpsimd.memset(m2, 0.0)
                    else:
                        nc.gpsimd.memset(pts[64:128, 0:64], 0.0)
                    pt_tiles[t] = pts

                # ---- PV: O'[q, 0:128]=O, O'[q, 128]=denominator ----
                ov = o_ps.tile([128, VW], f32, tag="ov")
                nmm = 1 + (1 if t >= 1 else 0) + (1 if t >= 2 else 0)
                nc.tensor.matmul(ov[:], ptv[:, qv], vbt(0),
                                 start=True, stop=(nmm == 1))
                if t >= 2:
                    nc.tensor.matmul(ov[:], pt_tiles[t - 1][:, 128:256],
                                     vbt(t - 1), start=False, stop=False)
                if t >= 1:
                    nc.tensor.matmul(ov[:], pt_tiles[t][:, 0:128],
                                     vbt(t), start=False, stop=True)

                # ---- normalize; store every STORE_W iterations ----
                sj = t % STORE_W
                if sj == 0:
                    osb = o_pool.tile([128, 128 * STORE_W], f32, tag="osb")
                recip = sums.tile([128, 1], f32, tag="recip")
                nc.vector.reciprocal(recip[:], ov[:, 128:129])
                nc.vector.tensor_scalar_mul(osb[:, 128 * sj:128 * sj + 128],
                                            ov[:, 0:128], recip[:])
                if sj == STORE_W - 1:
                    t0 = t - STORE_W + 1
                    oap = bass.AP(tensor=o_d[:].tensor,
                                  offset=128 * t0 * 128,
                                  ap=[[128, 128], [128 * 128, STORE_W],
                                      [1, 128]])
                    nc.sync.dma_start(out=oap, in_=osb[:])

    nc.compile()
    return nc


def _get_nc():
    global _CACHED_NC
    if _CACHED_NC is None:
        _CACHED_NC = _build_nc()
    return _CACHED_NC


def _run(inputs, trace=False, trace_kwargs=None):
    import ml_dtypes
    from concourse.bass_utils import run_bass_kernel_spmd

    q, k, v = inputs["q"], inputs["k"], inputs["v"]
    block_mask = np.asarray(inputs["block_mask"])
    assert np.array_equal(block_mask, _expected_block_mask()), \
        "kernel compiled for the DKernel predefined sparse pattern only"

    nc = _get_nc()
    f16 = ml_dtypes.float16 if hasattr(ml_dtypes, "float16") else np.float16
    in_maps = []
    for h in range(N_CORES):
        qh = np.asarray(q[0, :, h, :], dtype=np.float32)
        kh = np.asarray(k[0, :, h, :], dtype=np.float32)
        vh = np.asarray(v[0, :, h, :], dtype=np.float32)
        # pre-tiled [V | 1] in [128, 32*129] layout: tile t holds V rows
        # [128t, 128t+128) with a trailing ones column
        vt = np.ones((128, TILES * VW), dtype=np.float16)
        vr = vh.astype(np.float16).reshape(TILES, 128, D)
        for t in range(TILES):
            vt[:, VW * t:VW * t + 128] = vr[t]
        in_maps.append({
            "qT": np.ascontiguousarray(qh.T.astype(np.float16)),
            "kT": np.ascontiguousarray(kh.T.astype(np.float16)),
            "vt": vt,
        })
    kwargs = {}
    if trace:
        kwargs["trace"] = True
        if trace_kwargs:
            kwargs.update(trace_kwargs)
    res = run_bass_kernel_spmd(nc, in_maps, list(range(N_CORES)), **kwargs)
    out = np.empty((1, SEQ, N_HEADS, D), dtype=np.float32)
    for h in range(N_CORES):
        out[0, :, h, :] = res.results[h]["o"]
    return out, res


def kernel(q, k, v, block_mask):
    out, _ = _run({"q": q, "k": k, "v": v, "block_mask": block_mask})
    return out
